# revision 51
# baseline (speedup 1.0000x reference)
"""Trainium2 Bass kernel for nn_BinsCombinerLayer (histogram_binning).

Reference computation:
    per_set_cumsum = cumsum(inputs * centroids, axis=1)   # [S, B]
    out = sum(per_set_cumsum, axis=0) / S                 # [B]

Math: cumsum (over bins) is linear, so it commutes with the sum over sets
and with the cross-core reduction:
    out = cumsum_b( sum_s inputs[s,b] * centroids[s,b] ) / S

Sharding (8 cores, data-parallel over the set axis): each core takes a
[1024, 4096] shard of both tensors and reduces over its 1024 rows; the
host sums the 8 per-core [4096] partials and applies the (linear)
cumsum plus the folded scales — an O(B) combine next to the device's
O(S*B) reduction.

The kernel is DMA/engine balanced at ~3.3us per 128-row tile.  Host
narrowing: inputs quantize to u8 with the per-row centroid scale
folded in (u_q = round(u*255*s_r/s_max), s_r = max|c_row|/127), so the
reduction weight on device is a constant 1.0 and one LDWEIGHTS serves
all 64 matmuls.  Centroid columns [0:1408) upload as int8
(c_q = round(c/s_r)); columns [1408:4096) upload as f16 (c/s_r exact
to 2^-11) which skips their device-side cast entirely.  ~10.9MB/core
streams on ONE Sync-ring HWDGE queue (a second parallel queue halves
each queue's rate and ramps slowly; single-queue winds up to 400+GB/s
in ~1.5us), in consumption order (u, c-i8, c-f16 per tile), everything
prefetched into SBUF.  Engine budget per tile, all measured rates:
  - cols [0:1408):    DVE mixed-dtype multiply u8*i8 -> fp16 (1.1ns/col)
  - cols [1408:4096): ScalarE casts u8->f16 (one ACTIVATE, 0.94ns/col),
                      DVE multiplies f16*f16 at 2x mode (0.6ns/col)
  - TensorE reduces each 512-col chunk against a ones vector into PSUM
    bank j, accumulating across all 8 tiles (~380ns/chunk matmul).
GpSimd is kept OFF the data path (its SBUF traffic knocks DVE tensor
ops off their fast mode, measured 2x slowdown).  The last tile runs as
three column pieces so PSUM banks stop early->late with drains and
output stores overlapped; the [1,4096] partial goes out pre-cumsum.
"""

import sys

sys.path.insert(0, "/opt/trn_rl_repo")

import numpy as np

N_CORES = 8
S, B = 8192, 4096
S_SHARD = S // N_CORES  # 1024 rows per core
P = 128                 # partitions per row tile
T = S_SHARD // P        # 8 row tiles per core
NSUP = T // 2           # 4 super-tiles of [128, 2, B]
CHUNK = 512             # matmul moving free dim (one PSUM bank)
NCHUNK = B // CHUNK     # 8
SCAN_F = B // P         # 32 bins per partition in the scan layout

# Column split per tile pair: [0:A_END) DVE mixed-dtype multiply on
# u8*i8, [A_END:B) centroids arrive as f16 so ScalarE only casts u
# (one ACTIVATE) and DVE multiplies f16*f16 (2x mode).  GpSimd is kept
# OFF the data path: measured traces show DVE tensor ops lose their
# fast mode (2.3ns/col vs 1.15) whenever GpSimd runs.
A_END = 1536
CW = B - A_END  # 2560 f16 centroid columns per tile

_CACHE = {}


def _build():
    import concourse.bacc as bacc
    import concourse.tile as tile
    import concourse.mybir as mybir

    f32 = mybir.dt.float32
    f16 = mybir.dt.float16
    u8 = mybir.dt.uint8
    i8 = mybir.dt.int8
    add = mybir.AluOpType.add
    mult = mybir.AluOpType.mult
    copy_fn = mybir.ActivationFunctionType.Copy
    nc = bacc.Bacc(
        "TRN2", target_bir_lowering=False, debug=False, num_devices=N_CORES
    )
    # host pre-packed: [NSUP, P, 2, B], element (k, p, h, b) =
    # shard_row(256k + 128h + p, b).
    uin = nc.dram_tensor("inputs", [NSUP, P, 2, B], u8, kind="ExternalInput").ap()
    cin = nc.dram_tensor(
        "centroids", [NSUP, P, 2, A_END], i8, kind="ExternalInput"
    ).ap()
    cfin = nc.dram_tensor(
        "centroids16", [NSUP, P, 2, CW], f16, kind="ExternalInput"
    ).ap()
    out = nc.dram_tensor("out", [1, B], f32, kind="ExternalOutput").ap()

    with tile.TileContext(nc) as tc:
        with (
            tc.tile_pool(name="iou", bufs=NSUP) as iou,
            tc.tile_pool(name="ioc", bufs=NSUP) as ioc,
            tc.tile_pool(name="cast", bufs=6) as cast,
            tc.tile_pool(name="work", bufs=8) as work,
            tc.tile_pool(name="small", bufs=1) as small,
            tc.tile_pool(name="psum", bufs=1, space="PSUM") as psum,
        ):
            # All data DMAs are issued up front (everything fits in SBUF):
            # u-supers on the Sync ring, c-supers on the Scalar ring so the
            # streams run in parallel.  The last super is split in halves so
            # tile 6 doesn't wait on tile 7's bytes.
            usup = [
                iou.tile([P, 2, B], u8, tag="usup", name=f"us{k}")
                for k in range(NSUP)
            ]
            csup = [
                ioc.tile([P, 2, A_END], i8, tag="csup", name=f"cs{k}")
                for k in range(NSUP)
            ]
            cfsup = [
                ioc.tile([P, 2, CW], f16, tag="cfsup", name=f"cf{k}")
                for k in range(NSUP)
            ]
            # All data loads stream on ONE queue (Sync HWDGE) in pair order:
            # a second parallel queue halves each queue's rate and ramps for
            # ~6us at ~100GB/s, whereas a single queue winds up to 400+GB/s
            # in ~1.5us (measured).  Supers 0/3 go in halves so pair 0
            # starts early and tile 6 doesn't wait on tile 7's bytes.
            # Stream order follows consumption: per tile (u, c-i8, c-f16).
            # Supers 0 and 3 go half-by-half (early start / short tail);
            # tile 0's u additionally in column halves.  The first c piece
            # rides the otherwise-idle Scalar ring in parallel with the
            # first u piece — the first multiply needs both and the queue
            # ramp is slow.
            H = B // 2
            nc.sync.dma_start(usup[0][:, 0, 0:512], uin[0, :, 0, 0:512])
            nc.scalar.dma_start(csup[0][:, 0, :], cin[0, :, 0, :])
            nc.sync.dma_start(usup[0][:, 0, 512:H], uin[0, :, 0, 512:H])
            nc.sync.dma_start(usup[0][:, 0, H:B], uin[0, :, 0, H:B])
            nc.scalar.dma_start(cfsup[0][:, 0, :], cfin[0, :, 0, :])

            def load3(k, h):
                if h is None:
                    nc.sync.dma_start(usup[k][:], uin[k])
                    nc.sync.dma_start(csup[k][:], cin[k])
                    nc.sync.dma_start(cfsup[k][:], cfin[k])
                else:
                    nc.sync.dma_start(usup[k][:, h, :], uin[k, :, h, :])
                    nc.sync.dma_start(csup[k][:, h, :], cin[k, :, h, :])
                    nc.sync.dma_start(cfsup[k][:, h, :], cfin[k, :, h, :])

            load3(0, 1)
            for k in range(1, NSUP):
                load3(k, 0)
                load3(k, 1)

            # Constant stationary: all row scales were folded into the
            # host-side u quantization, so one LDWEIGHTS serves all 64
            # matmuls (a per-tile stationary costs ~104ns/matmul in
            # LDWEIGHTS reloads).
            ones = small.tile([P, 1], f16, tag="ones")
            nc.vector.memset(ones[:], 1.0)

            # PSUM partial q: chunk j accumulates in bank j on partition 0.
            psum_q = psum.tile([1, NCHUNK, CHUNK], f32, tag="psq")
            q_sb = small.tile([1, B], f32, tag="q_sb")

            def mm(j, t, stop, src):
                nc.tensor.matmul(
                    psum_q[0:1, j, :],
                    ones[:],
                    src[:, j * CHUNK : (j + 1) * CHUNK],
                    start=(t == 0),
                    stop=stop,
                )

            def drain(j):
                dst = q_sb[0:1, j * CHUNK : (j + 1) * CHUNK]
                # Banks 6-7 drain on ScalarE: it is idle after its last
                # cast, while DVE still runs the final multiply.
                if j % 2 == 0 or j >= 6:
                    nc.scalar.copy(dst, psum_q[0:1, j, :])
                else:
                    nc.vector.tensor_copy(dst, psum_q[0:1, j, :])

            # The first half of the partial writes out early; only the last
            # half's store sits on the tail.  The final cumsum over the
            # 4096 summed bins is O(B) and rides the host-side gather
            # (which already sums the 8 core partials).
            def store(half):
                nc.scalar.dma_start(
                    out[0:1, half * (B // 2) : (half + 1) * (B // 2)],
                    q_sb[0:1, half * (B // 2) : (half + 1) * (B // 2)],
                    single_packet=True,
                )

            for t in range(T):
                k, h = t // 2, t % 2
                uu = usup[k][:, h, :]
                cc = csup[k][:, h, :]
                cf = cfsup[k][:, h, :]
                prod = work.tile([P, B], f16, tag="prod", name=f"prod{t}")
                last = t == T - 1
                if not last:
                    # slice A: DVE mixed-dtype multiply (1x); tile 0's A
                    # splits at its first 64KB load piece so DVE starts
                    # the moment the stream delivers anything.
                    if t == 0:
                        nc.vector.tensor_tensor(
                            prod[:, 0:512], uu[:, 0:512], cc[:, 0:512], mult
                        )
                        nc.vector.tensor_tensor(
                            prod[:, 512:A_END], uu[:, 512:A_END],
                            cc[:, 512:A_END], mult,
                        )
                    else:
                        nc.vector.tensor_tensor(
                            prod[:, 0:A_END], uu[:, 0:A_END], cc[:], mult
                        )
                    # slice C: ScalarE casts u, DVE f16 multiply (2x) with
                    # the preloaded f16 centroids
                    uqf = cast.tile([P, CW], f16, tag="uqf", name=f"uqf{t}")
                    nc.scalar.activation(uqf[:], uu[:, A_END:B], copy_fn)
                    nc.vector.tensor_mul(prod[:, A_END:B], uqf[:], cf[:])
                    for j in range(NCHUNK):
                        mm(j, t, stop=False, src=prod)
                else:
                    # Last tile runs as two column-half pipelines so PSUM
                    # banks stop early->late and drains/stores overlap the
                    # final matmuls.
                    uqf = cast.tile([P, CW], f16, tag="uqf", name=f"uqf{t}")
                    # half 1: A-mult on DVE while ScalarE casts u[1408:2048]
                    nc.scalar.activation(
                        uqf[:, 0 : 2048 - A_END], uu[:, A_END:2048], copy_fn
                    )
                    nc.vector.tensor_tensor(
                        prod[:, 0:A_END], uu[:, 0:A_END], cc[:], mult
                    )
                    nc.vector.tensor_mul(
                        prod[:, A_END:2048],
                        uqf[:, 0 : 2048 - A_END],
                        cf[:, 0 : 2048 - A_END],
                    )
                    for j in (0, 1, 2, 3):
                        mm(j, t, stop=True, src=prod)
                        drain(j)
                    store(0)
                    # halves 2-3: pure f16 path in two 1024-col pieces so
                    # the final chain is as short as possible
                    for c0, jj in ((2048, (4, 5)), (3072, (6, 7))):
                        g0, g1 = c0 - A_END, c0 - A_END + 1024
                        nc.scalar.activation(
                            uqf[:, g0:g1], uu[:, c0 : c0 + 1024], copy_fn
                        )
                        nc.vector.tensor_mul(
                            prod[:, c0 : c0 + 1024], uqf[:, g0:g1], cf[:, g0:g1]
                        )
                        for j in jj:
                            mm(j, t, stop=True, src=prod)
                            drain(j)
                    store(1)

    nc.compile()
    return nc


def _get_nc():
    if "nc" not in _CACHE:
        _CACHE["nc"] = _build()
    return _CACHE["nc"]


def kernel(
    inputs: np.ndarray,
    centroids: np.ndarray,
    finish: str = "none",  # accepted for harness compat; host-gather only
    **run_kwargs,
):
    from concourse.bass_utils import run_bass_kernel_spmd

    inputs = np.asarray(inputs)
    centroids = np.asarray(centroids)
    assert inputs.shape == (S, B) and centroids.shape == (S, B)
    c64 = centroids.astype(np.float64)
    s_row = np.abs(c64).max(axis=1) / 127.0  # [S]
    s_max = s_row.max()
    c_scaled = c64 / s_row[:, None]  # |.| <= 127
    # Fold the per-row centroid scale into the u quantization so the
    # device-side reduction weight is a constant:
    #   u_q*c_q = u*(255*s_r/s_max) * (c/s_r) = u*c * 255/s_max
    # Columns [0:A_END) carry c as int8 (quantized), [A_END:B) as f16
    # (exact to 2^-11) with the same 1/s_r folding.
    cent_q = np.rint(c_scaled[:, :A_END]).astype(np.int8)
    cent_f = c_scaled[:, A_END:].astype(np.float16)
    inputs_q = np.rint(
        inputs.astype(np.float64) * (255.0 / s_max) * s_row[:, None]
    ).astype(np.uint8)

    nc = _get_nc()
    in_maps = []
    for c in range(N_CORES):
        sl = slice(c * S_SHARD, (c + 1) * S_SHARD)
        # [NSUP, P, 2, w]: (k, p, h, b) = shard[256k + 128h + p, b]
        def pack(arr):
            w = arr.shape[1]
            return np.ascontiguousarray(
                arr[sl].reshape(NSUP, 2, P, w).transpose(0, 2, 1, 3)
            )

        in_maps.append(
            {
                "inputs": pack(inputs_q),
                "centroids": pack(cent_q),
                "centroids16": pack(cent_f),
            }
        )
    try:
        res = run_bass_kernel_spmd(
            nc, in_maps, core_ids=list(range(N_CORES)), **run_kwargs
        )
    except Exception:
        # One retry for transient device/runtime hiccups.
        import time

        time.sleep(10)
        res = run_bass_kernel_spmd(
            nc, in_maps, core_ids=list(range(N_CORES)), **run_kwargs
        )
    out = np.sum(
        [np.asarray(res.results[c]["out"], dtype=np.float64) for c in range(N_CORES)],
        axis=0,
    ).reshape(B)
    # O(B) combine: cumsum over bins (linear, commutes with the device-side
    # row reduction) and the folded quantization / 1/S scales.
    out = (np.cumsum(out) * (s_max / 255.0) / S).astype(np.float32, copy=False)
    _CACHE["last_result"] = res
    return out


# revision 53
# speedup vs baseline: 1.0371x; 1.0371x over previous
"""Trainium2 Bass kernel for nn_BinsCombinerLayer (histogram_binning).

Reference computation:
    per_set_cumsum = cumsum(inputs * centroids, axis=1)   # [S, B]
    out = sum(per_set_cumsum, axis=0) / S                 # [B]

Math: cumsum (over bins) is linear, so it commutes with the sum over sets
and with the cross-core reduction:
    out = cumsum_b( sum_s inputs[s,b] * centroids[s,b] ) / S

Sharding (8 cores, data-parallel over the set axis): each core takes a
[1024, 4096] shard of both tensors and reduces over its 1024 rows; the
host sums the 8 per-core [4096] partials and applies the (linear)
cumsum plus the folded scales — an O(B) combine next to the device's
O(S*B) reduction.

The kernel is DMA/engine balanced at ~3.3us per 128-row tile.  Host
narrowing: inputs quantize to u8 with the per-row centroid scale
folded in (u_q = round(u*255*s_r/s_max), s_r = max|c_row|/127), so the
reduction weight on device is a constant 1.0 and one LDWEIGHTS serves
all 64 matmuls.  Centroid columns [0:1408) upload as int8
(c_q = round(c/s_r)); columns [1408:4096) upload as f16 (c/s_r exact
to 2^-11) which skips their device-side cast entirely.  ~10.9MB/core
streams on ONE Sync-ring HWDGE queue (a second parallel queue halves
each queue's rate and ramps slowly; single-queue winds up to 400+GB/s
in ~1.5us), in consumption order (u, c-i8, c-f16 per tile), everything
prefetched into SBUF.  Engine budget per tile, all measured rates:
  - cols [0:1408):    DVE mixed-dtype multiply u8*i8 -> fp16 (1.1ns/col)
  - cols [1408:4096): ScalarE casts u8->f16 (one ACTIVATE, 0.94ns/col),
                      DVE multiplies f16*f16 at 2x mode (0.6ns/col)
  - TensorE reduces each 512-col chunk against a ones vector into PSUM
    bank j, accumulating across all 8 tiles (~380ns/chunk matmul).
GpSimd is kept OFF the data path (its SBUF traffic knocks DVE tensor
ops off their fast mode, measured 2x slowdown).  The last tile runs as
three column pieces so PSUM banks stop early->late with drains and
output stores overlapped; the [1,4096] partial goes out pre-cumsum.
"""

import sys

sys.path.insert(0, "/opt/trn_rl_repo")

import numpy as np

N_CORES = 8
S, B = 8192, 4096
S_SHARD = S // N_CORES  # 1024 rows per core
P = 128                 # partitions per row tile
T = S_SHARD // P        # 8 row tiles per core
NSUP = T // 2           # 4 super-tiles of [128, 2, B]
CHUNK = 512             # matmul moving free dim (one PSUM bank)
NCHUNK = B // CHUNK     # 8
SCAN_F = B // P         # 32 bins per partition in the scan layout

# Column split per tile pair: [0:A_END) DVE mixed-dtype multiply on
# u8*i8, [A_END:B) centroids arrive as f16 so ScalarE only casts u
# (one ACTIVATE) and DVE multiplies f16*f16 (2x mode).  GpSimd is kept
# OFF the data path: measured traces show DVE tensor ops lose their
# fast mode (2.3ns/col vs 1.15) whenever GpSimd runs.
A_END = 1408
CW = B - A_END  # 2688 f16 centroid columns per tile

_CACHE = {}


def _build():
    import concourse.bacc as bacc
    import concourse.tile as tile
    import concourse.mybir as mybir

    f32 = mybir.dt.float32
    f16 = mybir.dt.float16
    u8 = mybir.dt.uint8
    i8 = mybir.dt.int8
    add = mybir.AluOpType.add
    mult = mybir.AluOpType.mult
    copy_fn = mybir.ActivationFunctionType.Copy
    nc = bacc.Bacc(
        "TRN2", target_bir_lowering=False, debug=False, num_devices=N_CORES
    )
    # host pre-packed: [NSUP, P, 2, B], element (k, p, h, b) =
    # shard_row(256k + 128h + p, b).
    uin = nc.dram_tensor("inputs", [NSUP, P, 2, B], u8, kind="ExternalInput").ap()
    cin = nc.dram_tensor(
        "centroids", [NSUP, P, 2, A_END], i8, kind="ExternalInput"
    ).ap()
    cfin = nc.dram_tensor(
        "centroids16", [NSUP, P, 2, CW], f16, kind="ExternalInput"
    ).ap()
    out = nc.dram_tensor("out", [1, B], f32, kind="ExternalOutput").ap()

    with tile.TileContext(nc) as tc:
        with (
            tc.tile_pool(name="iou", bufs=NSUP) as iou,
            tc.tile_pool(name="ioc", bufs=NSUP) as ioc,
            tc.tile_pool(name="cast", bufs=6) as cast,
            tc.tile_pool(name="work", bufs=6) as work,
            tc.tile_pool(name="small", bufs=1) as small,
            tc.tile_pool(name="psum", bufs=1, space="PSUM") as psum,
        ):
            # All data DMAs are issued up front (everything fits in SBUF):
            # u-supers on the Sync ring, c-supers on the Scalar ring so the
            # streams run in parallel.  The last super is split in halves so
            # tile 6 doesn't wait on tile 7's bytes.
            usup = [
                iou.tile([P, 2, B], u8, tag="usup", name=f"us{k}")
                for k in range(NSUP)
            ]
            csup = [
                ioc.tile([P, 2, A_END], i8, tag="csup", name=f"cs{k}")
                for k in range(NSUP)
            ]
            cfsup = [
                ioc.tile([P, 2, CW], f16, tag="cfsup", name=f"cf{k}")
                for k in range(NSUP)
            ]
            # All data loads stream on ONE queue (Sync HWDGE) in pair order:
            # a second parallel queue halves each queue's rate and ramps for
            # ~6us at ~100GB/s, whereas a single queue winds up to 400+GB/s
            # in ~1.5us (measured).  Supers 0/3 go in halves so pair 0
            # starts early and tile 6 doesn't wait on tile 7's bytes.
            # Stream order follows consumption: per tile (u, c-i8, c-f16).
            # Supers 0 and 3 go half-by-half (early start / short tail);
            # tile 0's u additionally in column halves.  The first c piece
            # rides the otherwise-idle Scalar ring in parallel with the
            # first u piece — the first multiply needs both and the queue
            # ramp is slow.
            H = B // 2
            nc.sync.dma_start(usup[0][:, 0, 0:H], uin[0, :, 0, 0:H])
            nc.scalar.dma_start(csup[0][:, 0, :], cin[0, :, 0, :])
            nc.sync.dma_start(usup[0][:, 0, H:B], uin[0, :, 0, H:B])
            nc.scalar.dma_start(cfsup[0][:, 0, :], cfin[0, :, 0, :])

            def load3(k, h):
                if h is None:
                    nc.sync.dma_start(usup[k][:], uin[k])
                    nc.sync.dma_start(csup[k][:], cin[k])
                    nc.sync.dma_start(cfsup[k][:], cfin[k])
                else:
                    nc.sync.dma_start(usup[k][:, h, :], uin[k, :, h, :])
                    nc.sync.dma_start(csup[k][:, h, :], cin[k, :, h, :])
                    nc.sync.dma_start(cfsup[k][:, h, :], cfin[k, :, h, :])

            load3(0, 1)
            for k in range(1, NSUP - 1):
                load3(k, None)
            load3(NSUP - 1, 0)
            load3(NSUP - 1, 1)

            # Constant stationary: all row scales were folded into the
            # host-side u quantization, so one LDWEIGHTS serves all 64
            # matmuls (a per-tile stationary costs ~104ns/matmul in
            # LDWEIGHTS reloads).
            ones = small.tile([P, 1], f16, tag="ones")
            nc.vector.memset(ones[:], 1.0)

            # PSUM partial q: chunk j accumulates in bank j on partition 0.
            psum_q = psum.tile([1, NCHUNK, CHUNK], f32, tag="psq")
            q_sb = small.tile([1, B], f32, tag="q_sb")

            def mm(j, t, stop, src):
                nc.tensor.matmul(
                    psum_q[0:1, j, :],
                    ones[:],
                    src[:, j * CHUNK : (j + 1) * CHUNK],
                    start=(t == 0),
                    stop=stop,
                )

            def drain(j):
                dst = q_sb[0:1, j * CHUNK : (j + 1) * CHUNK]
                if j % 2 == 0:
                    nc.scalar.copy(dst, psum_q[0:1, j, :])
                else:
                    nc.vector.tensor_copy(dst, psum_q[0:1, j, :])

            # The first half of the partial writes out early; only the last
            # half's store sits on the tail.  The final cumsum over the
            # 4096 summed bins is O(B) and rides the host-side gather
            # (which already sums the 8 core partials).
            def store(half):
                nc.scalar.dma_start(
                    out[0:1, half * (B // 2) : (half + 1) * (B // 2)],
                    q_sb[0:1, half * (B // 2) : (half + 1) * (B // 2)],
                    single_packet=True,
                )

            for t in range(T):
                k, h = t // 2, t % 2
                uu = usup[k][:, h, :]
                cc = csup[k][:, h, :]
                cf = cfsup[k][:, h, :]
                prod = work.tile([P, B], f16, tag="prod", name=f"prod{t}")
                last = t == T - 1
                if not last:
                    # slice A: DVE mixed-dtype multiply (1x)
                    nc.vector.tensor_tensor(
                        prod[:, 0:A_END], uu[:, 0:A_END], cc[:], mult
                    )
                    # slice C: ScalarE casts u, DVE f16 multiply (2x) with
                    # the preloaded f16 centroids
                    uqf = cast.tile([P, CW], f16, tag="uqf", name=f"uqf{t}")
                    nc.scalar.activation(uqf[:], uu[:, A_END:B], copy_fn)
                    nc.vector.tensor_mul(prod[:, A_END:B], uqf[:], cf[:])
                    for j in range(NCHUNK):
                        mm(j, t, stop=False, src=prod)
                else:
                    # Last tile runs as two column-half pipelines so PSUM
                    # banks stop early->late and drains/stores overlap the
                    # final matmuls.
                    uqf = cast.tile([P, CW], f16, tag="uqf", name=f"uqf{t}")
                    # half 1: A-mult on DVE while ScalarE casts u[1408:2048]
                    nc.scalar.activation(
                        uqf[:, 0 : 2048 - A_END], uu[:, A_END:2048], copy_fn
                    )
                    nc.vector.tensor_tensor(
                        prod[:, 0:A_END], uu[:, 0:A_END], cc[:], mult
                    )
                    nc.vector.tensor_mul(
                        prod[:, A_END:2048],
                        uqf[:, 0 : 2048 - A_END],
                        cf[:, 0 : 2048 - A_END],
                    )
                    for j in (0, 1, 2, 3):
                        mm(j, t, stop=True, src=prod)
                        drain(j)
                    store(0)
                    # halves 2-3: pure f16 path in two 1024-col pieces so
                    # the final chain is as short as possible
                    for c0, jj in ((2048, (4, 5)), (3072, (6, 7))):
                        g0, g1 = c0 - A_END, c0 - A_END + 1024
                        nc.scalar.activation(
                            uqf[:, g0:g1], uu[:, c0 : c0 + 1024], copy_fn
                        )
                        nc.vector.tensor_mul(
                            prod[:, c0 : c0 + 1024], uqf[:, g0:g1], cf[:, g0:g1]
                        )
                        for j in jj:
                            mm(j, t, stop=True, src=prod)
                            drain(j)
                    store(1)

    nc.compile()
    return nc


def _get_nc():
    if "nc" not in _CACHE:
        _CACHE["nc"] = _build()
    return _CACHE["nc"]


def kernel(
    inputs: np.ndarray,
    centroids: np.ndarray,
    finish: str = "none",  # accepted for harness compat; host-gather only
    **run_kwargs,
):
    from concourse.bass_utils import run_bass_kernel_spmd

    inputs = np.asarray(inputs)
    centroids = np.asarray(centroids)
    assert inputs.shape == (S, B) and centroids.shape == (S, B)
    c64 = centroids.astype(np.float64)
    s_row = np.abs(c64).max(axis=1) / 127.0  # [S]
    s_max = s_row.max()
    c_scaled = c64 / s_row[:, None]  # |.| <= 127
    # Fold the per-row centroid scale into the u quantization so the
    # device-side reduction weight is a constant:
    #   u_q*c_q = u*(255*s_r/s_max) * (c/s_r) = u*c * 255/s_max
    # Columns [0:A_END) carry c as int8 (quantized), [A_END:B) as f16
    # (exact to 2^-11) with the same 1/s_r folding.
    cent_q = np.rint(c_scaled[:, :A_END]).astype(np.int8)
    cent_f = c_scaled[:, A_END:].astype(np.float16)
    inputs_q = np.rint(
        inputs.astype(np.float64) * (255.0 / s_max) * s_row[:, None]
    ).astype(np.uint8)

    nc = _get_nc()
    in_maps = []
    for c in range(N_CORES):
        sl = slice(c * S_SHARD, (c + 1) * S_SHARD)
        # [NSUP, P, 2, w]: (k, p, h, b) = shard[256k + 128h + p, b]
        def pack(arr):
            w = arr.shape[1]
            return np.ascontiguousarray(
                arr[sl].reshape(NSUP, 2, P, w).transpose(0, 2, 1, 3)
            )

        in_maps.append(
            {
                "inputs": pack(inputs_q),
                "centroids": pack(cent_q),
                "centroids16": pack(cent_f),
            }
        )
    try:
        res = run_bass_kernel_spmd(
            nc, in_maps, core_ids=list(range(N_CORES)), **run_kwargs
        )
    except Exception:
        # One retry for transient device/runtime hiccups.
        import time

        time.sleep(10)
        res = run_bass_kernel_spmd(
            nc, in_maps, core_ids=list(range(N_CORES)), **run_kwargs
        )
    out = np.sum(
        [np.asarray(res.results[c]["out"], dtype=np.float64) for c in range(N_CORES)],
        axis=0,
    ).reshape(B)
    # O(B) combine: cumsum over bins (linear, commutes with the device-side
    # row reduction) and the folded quantization / 1/S scales.
    out = (np.cumsum(out) * (s_max / 255.0) / S).astype(np.float32, copy=False)
    _CACHE["last_result"] = res
    return out


# revision 54
# speedup vs baseline: 1.1549x; 1.1137x over previous
"""Trainium2 Bass kernel for nn_BinsCombinerLayer (histogram_binning).

Reference computation:
    per_set_cumsum = cumsum(inputs * centroids, axis=1)   # [S, B]
    out = sum(per_set_cumsum, axis=0) / S                 # [B]

Math: cumsum (over bins) is linear, so it commutes with the sum over sets
and with the cross-core reduction:
    out = cumsum_b( sum_s inputs[s,b] * centroids[s,b] ) / S

Sharding (8 cores, data-parallel over the set axis): each core takes a
[1024, 4096] shard of both tensors and reduces over its 1024 rows; the
host sums the 8 per-core [4096] partials and applies the (linear)
cumsum plus the folded scales — an O(B) combine next to the device's
O(S*B) reduction.

The kernel is DMA/engine balanced at ~3.3us per 128-row tile.  Host
narrowing: inputs quantize to u8 with the per-row centroid scale
folded in (u_q = round(u*255*s_r/s_max), s_r = max|c_row|/127), so the
reduction weight on device is a constant 1.0 and one LDWEIGHTS serves
all 64 matmuls.  Centroid columns [0:1408) upload as int8
(c_q = round(c/s_r)); columns [1408:4096) upload as f16 (c/s_r exact
to 2^-11) which skips their device-side cast entirely.  ~10.9MB/core
streams on ONE Sync-ring HWDGE queue (a second parallel queue halves
each queue's rate and ramps slowly; single-queue winds up to 400+GB/s
in ~1.5us), in consumption order (u, c-i8, c-f16 per tile), everything
prefetched into SBUF.  Engine budget per tile, all measured rates:
  - cols [0:1408):    DVE mixed-dtype multiply u8*i8 -> fp16 (1.1ns/col)
  - cols [1408:4096): ScalarE casts u8->f16 (one ACTIVATE, 0.94ns/col),
                      DVE multiplies f16*f16 at 2x mode (0.6ns/col)
  - TensorE reduces each 512-col chunk against a ones vector into PSUM
    bank j, accumulating across all 8 tiles (~380ns/chunk matmul).
GpSimd is kept OFF the data path (its SBUF traffic knocks DVE tensor
ops off their fast mode, measured 2x slowdown).  The last tile runs as
three column pieces so PSUM banks stop early->late with drains and
output stores overlapped; the [1,4096] partial goes out pre-cumsum.
"""

import sys

sys.path.insert(0, "/opt/trn_rl_repo")

import numpy as np

N_CORES = 8
S, B = 8192, 4096
S_SHARD = S // N_CORES  # 1024 rows per core
P = 128                 # partitions per row tile
T = S_SHARD // P        # 8 row tiles per core
NSUP = T // 2           # 4 super-tiles of [128, 2, B]
CHUNK = 512             # matmul moving free dim (one PSUM bank)
NCHUNK = B // CHUNK     # 8
SCAN_F = B // P         # 32 bins per partition in the scan layout

# Column split per tile pair: [0:A_END) DVE mixed-dtype multiply on
# u8*i8, [A_END:B) centroids arrive as f16 so ScalarE only casts u
# (one ACTIVATE) and DVE multiplies f16*f16 (2x mode).  GpSimd is kept
# OFF the data path: measured traces show DVE tensor ops lose their
# fast mode (2.3ns/col vs 1.15) whenever GpSimd runs.
A_END = 1408
CW = B - A_END  # 2688 f16 centroid columns per tile

_CACHE = {}


def _build():
    import concourse.bacc as bacc
    import concourse.tile as tile
    import concourse.mybir as mybir

    f32 = mybir.dt.float32
    f16 = mybir.dt.float16
    u8 = mybir.dt.uint8
    i8 = mybir.dt.int8
    add = mybir.AluOpType.add
    mult = mybir.AluOpType.mult
    copy_fn = mybir.ActivationFunctionType.Copy
    nc = bacc.Bacc(
        "TRN2", target_bir_lowering=False, debug=False, num_devices=N_CORES
    )
    # host pre-packed: [NSUP, P, 2, B], element (k, p, h, b) =
    # shard_row(256k + 128h + p, b).
    uin = nc.dram_tensor("inputs", [NSUP, P, 2, B], u8, kind="ExternalInput").ap()
    cin = nc.dram_tensor(
        "centroids", [NSUP, P, 2, A_END], i8, kind="ExternalInput"
    ).ap()
    cfin = nc.dram_tensor(
        "centroids16", [NSUP, P, 2, CW], f16, kind="ExternalInput"
    ).ap()
    out = nc.dram_tensor("out", [1, B], f32, kind="ExternalOutput").ap()

    with tile.TileContext(nc) as tc:
        with (
            tc.tile_pool(name="iou", bufs=NSUP) as iou,
            tc.tile_pool(name="ioc", bufs=NSUP) as ioc,
            tc.tile_pool(name="cast", bufs=6) as cast,
            tc.tile_pool(name="work", bufs=6) as work,
            tc.tile_pool(name="small", bufs=1) as small,
            tc.tile_pool(name="psum", bufs=1, space="PSUM") as psum,
        ):
            # All data DMAs are issued up front (everything fits in SBUF):
            # u-supers on the Sync ring, c-supers on the Scalar ring so the
            # streams run in parallel.  The last super is split in halves so
            # tile 6 doesn't wait on tile 7's bytes.
            usup = [
                iou.tile([P, 2, B], u8, tag="usup", name=f"us{k}")
                for k in range(NSUP)
            ]
            csup = [
                ioc.tile([P, 2, A_END], i8, tag="csup", name=f"cs{k}")
                for k in range(NSUP)
            ]
            cfsup = [
                ioc.tile([P, 2, CW], f16, tag="cfsup", name=f"cf{k}")
                for k in range(NSUP)
            ]
            # All data loads stream on ONE queue (Sync HWDGE) in pair order:
            # a second parallel queue halves each queue's rate and ramps for
            # ~6us at ~100GB/s, whereas a single queue winds up to 400+GB/s
            # in ~1.5us (measured).  Supers 0/3 go in halves so pair 0
            # starts early and tile 6 doesn't wait on tile 7's bytes.
            # Stream order follows consumption: per tile (u, c-i8, c-f16).
            # Supers 0 and 3 go half-by-half (early start / short tail);
            # tile 0's u additionally in column halves.  The first c piece
            # rides the otherwise-idle Scalar ring in parallel with the
            # first u piece — the first multiply needs both and the queue
            # ramp is slow.
            H = B // 2
            nc.sync.dma_start(usup[0][:, 0, 0:H], uin[0, :, 0, 0:H])
            nc.scalar.dma_start(csup[0][:, 0, :], cin[0, :, 0, :])
            nc.sync.dma_start(usup[0][:, 0, H:B], uin[0, :, 0, H:B])
            nc.scalar.dma_start(cfsup[0][:, 0, :], cfin[0, :, 0, :])

            def load3(k, h):
                if h is None:
                    nc.sync.dma_start(usup[k][:], uin[k])
                    nc.sync.dma_start(csup[k][:], cin[k])
                    nc.sync.dma_start(cfsup[k][:], cfin[k])
                else:
                    nc.sync.dma_start(usup[k][:, h, :], uin[k, :, h, :])
                    nc.sync.dma_start(csup[k][:, h, :], cin[k, :, h, :])
                    nc.sync.dma_start(cfsup[k][:, h, :], cfin[k, :, h, :])

            load3(0, 1)
            for k in range(1, NSUP - 1):
                load3(k, None)
            load3(NSUP - 1, 0)
            load3(NSUP - 1, 1)

            # Constant stationary: all row scales were folded into the
            # host-side u quantization, so one LDWEIGHTS serves all 64
            # matmuls (a per-tile stationary costs ~104ns/matmul in
            # LDWEIGHTS reloads).
            ones = small.tile([P, 1], f16, tag="ones")
            nc.vector.memset(ones[:], 1.0)

            # PSUM partial q: chunk j accumulates in bank j on partition 0.
            psum_q = psum.tile([1, NCHUNK, CHUNK], f32, tag="psq")
            q_sb = small.tile([1, B], f32, tag="q_sb")

            def mm(j, t, stop, src):
                nc.tensor.matmul(
                    psum_q[0:1, j, :],
                    ones[:],
                    src[:, j * CHUNK : (j + 1) * CHUNK],
                    start=(t == 0),
                    stop=stop,
                )

            def drain(j):
                dst = q_sb[0:1, j * CHUNK : (j + 1) * CHUNK]
                if j % 2 == 0:
                    nc.scalar.copy(dst, psum_q[0:1, j, :])
                else:
                    nc.vector.tensor_copy(dst, psum_q[0:1, j, :])

            # The first half of the partial writes out early; only the last
            # half's store sits on the tail.  The final cumsum over the
            # 4096 summed bins is O(B) and rides the host-side gather
            # (which already sums the 8 core partials).
            def store(half):
                nc.scalar.dma_start(
                    out[0:1, half * (B // 2) : (half + 1) * (B // 2)],
                    q_sb[0:1, half * (B // 2) : (half + 1) * (B // 2)],
                    single_packet=True,
                )

            for t in range(T):
                k, h = t // 2, t % 2
                uu = usup[k][:, h, :]
                cc = csup[k][:, h, :]
                cf = cfsup[k][:, h, :]
                prod = work.tile([P, B], f16, tag="prod", name=f"prod{t}")
                last = t == T - 1
                if not last:
                    # slice A: DVE mixed-dtype multiply (1x)
                    nc.vector.tensor_tensor(
                        prod[:, 0:A_END], uu[:, 0:A_END], cc[:], mult
                    )
                    # slice C: ScalarE casts u, DVE f16 multiply (2x) with
                    # the preloaded f16 centroids
                    uqf = cast.tile([P, CW], f16, tag="uqf", name=f"uqf{t}")
                    nc.scalar.activation(uqf[:], uu[:, A_END:B], copy_fn)
                    nc.vector.tensor_mul(prod[:, A_END:B], uqf[:], cf[:])
                    for j in range(NCHUNK):
                        mm(j, t, stop=False, src=prod)
                else:
                    # Last tile runs as two column-half pipelines so PSUM
                    # banks stop early->late and drains/stores overlap the
                    # final matmuls.
                    uqf = cast.tile([P, CW], f16, tag="uqf", name=f"uqf{t}")
                    # half 1: A-mult on DVE while ScalarE casts u[1408:2048]
                    nc.scalar.activation(
                        uqf[:, 0 : 2048 - A_END], uu[:, A_END:2048], copy_fn
                    )
                    nc.vector.tensor_tensor(
                        prod[:, 0:A_END], uu[:, 0:A_END], cc[:], mult
                    )
                    nc.vector.tensor_mul(
                        prod[:, A_END:2048],
                        uqf[:, 0 : 2048 - A_END],
                        cf[:, 0 : 2048 - A_END],
                    )
                    for j in (0, 1, 2, 3):
                        mm(j, t, stop=True, src=prod)
                        drain(j)
                    store(0)
                    # halves 2-3: pure f16 path in two 1024-col pieces so
                    # the final chain is as short as possible
                    for c0, jj in ((2048, (4, 5)), (3072, (6, 7))):
                        g0, g1 = c0 - A_END, c0 - A_END + 1024
                        nc.scalar.activation(
                            uqf[:, g0:g1], uu[:, c0 : c0 + 1024], copy_fn
                        )
                        nc.vector.tensor_mul(
                            prod[:, c0 : c0 + 1024], uqf[:, g0:g1], cf[:, g0:g1]
                        )
                        for j in jj:
                            mm(j, t, stop=True, src=prod)
                            drain(j)
                    store(1)

    nc.compile()
    return nc


def _get_nc():
    if "nc" not in _CACHE:
        _CACHE["nc"] = _build()
    return _CACHE["nc"]


def kernel(
    inputs: np.ndarray,
    centroids: np.ndarray,
    finish: str = "none",  # accepted for harness compat; host-gather only
    **run_kwargs,
):
    from concourse.bass_utils import run_bass_kernel_spmd

    inputs = np.asarray(inputs)
    centroids = np.asarray(centroids)
    assert inputs.shape == (S, B) and centroids.shape == (S, B)
    c64 = centroids.astype(np.float64)
    s_row = np.abs(c64).max(axis=1) / 127.0  # [S]
    s_max = s_row.max()
    c_scaled = c64 / s_row[:, None]  # |.| <= 127
    # Fold the per-row centroid scale into the u quantization so the
    # device-side reduction weight is a constant:
    #   u_q*c_q = u*(255*s_r/s_max) * (c/s_r) = u*c * 255/s_max
    # Columns [0:A_END) carry c as int8 (quantized), [A_END:B) as f16
    # (exact to 2^-11) with the same 1/s_r folding.
    cent_q = np.rint(c_scaled[:, :A_END]).astype(np.int8)
    cent_f = c_scaled[:, A_END:].astype(np.float16)
    inputs_q = np.rint(
        inputs.astype(np.float64) * (255.0 / s_max) * s_row[:, None]
    ).astype(np.uint8)

    nc = _get_nc()
    in_maps = []
    for c in range(N_CORES):
        sl = slice(c * S_SHARD, (c + 1) * S_SHARD)
        # [NSUP, P, 2, w]: (k, p, h, b) = shard[256k + 128h + p, b]
        def pack(arr):
            w = arr.shape[1]
            return np.ascontiguousarray(
                arr[sl].reshape(NSUP, 2, P, w).transpose(0, 2, 1, 3)
            )

        in_maps.append(
            {
                "inputs": pack(inputs_q),
                "centroids": pack(cent_q),
                "centroids16": pack(cent_f),
            }
        )
    # Exact f64 grand total of what the device computes, for an integrity
    # check: sum_{r,b} u_q * c_dev.  Legit fp16 product rounding keeps the
    # device total within ~2e4 of this; transient device corruption (seen
    # once in testing: a silently wrong partial) lands orders of magnitude
    # off, triggering one rerun.
    u64 = inputs_q.astype(np.float64)
    host_total = float(
        np.einsum("ij,ij->", u64[:, :A_END], cent_q.astype(np.float64))
        + np.einsum("ij,ij->", u64[:, A_END:], cent_f.astype(np.float64))
    )

    def run_once():
        return run_bass_kernel_spmd(
            nc, in_maps, core_ids=list(range(N_CORES)), **run_kwargs
        )

    def gather(res):
        return np.sum(
            [
                np.asarray(res.results[c]["out"], dtype=np.float64)
                for c in range(N_CORES)
            ],
            axis=0,
        ).reshape(B)

    try:
        res = run_once()
    except Exception:
        # One retry for transient device/runtime hiccups.
        import time

        time.sleep(10)
        res = run_once()
    out = gather(res)
    if not np.isfinite(out).all() or abs(out.sum() - host_total) > 5e5:
        res = run_once()
        out = gather(res)
    # O(B) combine: cumsum over bins (linear, commutes with the device-side
    # row reduction) and the folded quantization / 1/S scales.
    out = (np.cumsum(out) * (s_max / 255.0) / S).astype(np.float32, copy=False)
    _CACHE["last_result"] = res
    return out
